# revision 35
# baseline (speedup 1.0000x reference)
"""Cross-attention kernel for Trainium2 (8 NeuronCores).

Problem: nn_Attention (B=4, N_LAT=512, N_CTX=4096, DIM=512, HEADS=8, DIM_HEAD=64)
  ctx = concat([x, context], axis=1)            [b, 4608, 512]
  q = x @ Wq.T ; k,v = split(ctx @ Wkv.T)
  out = softmax(q k^T / 8) v  per (b, head), then @ Wout.T

Sharding: 8 cores = 4 batches x 2 head-groups (4 heads each).
Each core computes its batch's attention for its 4 heads plus the partial
output projection; the host sums the two partials per batch.

Per-core dataflow:
  - scores computed TRANSPOSED: sT[j, i] = k[j,:] . q[i,:] so the softmax
    denominator and the attention*V matmul run on the TensorEngine with no
    transposes. exp has no max-subtraction (fp32/bf16 exp cannot overflow
    at |s/8| <= ~10), normalization at the end.
  - v is augmented with a ones column, so U[64,:] accumulates the softmax
    denominator for free during the A*V matmul.
  - the softmax exp (the dominant engine cost: 9.4M elements/core, and only
    the Activation engine has exp) is SPLIT across two engines: most first
    waves run on the Vector engine via a custom DVE op implementing
    Schraudolph's bit-trick exp (int16(round(128*log2e*x + 128*(127-c)))
    bit-viewed as bf16), the rest on the Activation engine's exact exp.
  - projection matmuls are emitted BETWEEN a chunk's exps and its A*V
    matmuls: the PE queue is FIFO, so this independent work fills the
    window where A*V waits on the engines' exp.
  - projections and scores in float32r/bf16 (1 cyc/row); exp output, v and
    the A*V matmul in bf16.
"""

import ml_dtypes
import numpy as np

import concourse.bass as bass
import concourse.mybir as mybir
import concourse.tile as tile
from concourse import bacc, bass_utils, dve_ops
from concourse.dve_spec import Spec, Src0, C0, C1, relu, lower
from concourse.dve_uop import DveOpSpec

F32 = mybir.dt.float32
F32R = mybir.dt.float32r
BF16 = mybir.dt.bfloat16
I16 = mybir.dt.int16
EXP = mybir.ActivationFunctionType.Exp
LOG2E = float(np.log2(np.e))
SCH_C = 0.043

B = 4
NI = 512         # query tokens per batch
NJ = 4608        # key/value tokens (x ++ context)
D = 512          # model dim
E = 256          # head-group inner dim (4 heads x 64)
DH = 64
NBLK = 9         # j-blocks of 512
NJC = 36         # j-chunks of 128
VW = DH + 1      # v block width per (chunk, head): 64 cols of v + ones col
SCALE = float(DH) ** -0.5

_CACHE = {}


def _register_exp_op():
    """Custom DVE op: out = relu(in*C0 + C1) written via int16 conversion.
    With C0 = 128*log2e*scale, C1 = 128*(127-c), the int16 bit pattern is the
    bf16 encoding of exp(in*scale) (Schraudolph's approximation, ~1.7% rms)."""
    for op in dve_ops.OPS:
        if op.name == "EXP_SCH_ANT":
            return op
    spec = Spec(
        body=relu(Src0 * C0 + C1),
        reference=lambda in0, in1, s0, s1, imm2: np.maximum(in0 * s0 + s1, 0.0).astype(
            np.float32
        ),
    )
    shas = {}
    for ver in ("v3", "v4"):
        tmp = DveOpSpec(name="EXP_SCH_ANT", opcode=0, uops=lower(spec, ver=ver), rd1_en=False)
        shas[ver] = tmp.sha(ver)
    op = dve_ops.DveOp("EXP_SCH_ANT", spec, subdim=False, uops_sha=shas)
    dve_ops.OPS.append(op)
    dve_ops.CUSTOM_DVE_SPECS[op.name] = op.spec
    dve_ops._SUB_OPCODE_FOR_NAME[op.name] = max(dve_ops._SUB_OPCODE_FOR_NAME.values()) + 1
    return op


def _build_nc(reps: int = 1, rep_epilogue: bool = True, et_bufs: int = 4, cx_bufs: int = 2,
              dve_exp: bool = True, act_every: int = 6):
    exp_op = _register_exp_op() if dve_exp else None
    nc = bacc.Bacc("TRN2", target_bir_lowering=False, debug=False, num_devices=8)
    xT_d = nc.dram_tensor("xT", [D, NI], F32, kind="ExternalInput").ap()
    ctxB_d = nc.dram_tensor("ctxB", [D, NJ], BF16, kind="ExternalInput").ap()
    wqT_d = nc.dram_tensor("wqT", [D, E], F32, kind="ExternalInput").ap()
    wkB_d = nc.dram_tensor("wkB", [D, E], BF16, kind="ExternalInput").ap()
    wvB_d = nc.dram_tensor("wvB", [D, E], BF16, kind="ExternalInput").ap()
    woT_d = nc.dram_tensor("woT", [4, DH, D], F32, kind="ExternalInput").ap()
    out_d = nc.dram_tensor("out", [NI, D], F32, kind="ExternalOutput").ap()

    with tile.TileContext(nc) as tc:
        with (
            tc.tile_pool(name="persist", bufs=1) as pp,
            tc.tile_pool(name="stream", bufs=cx_bufs) as sp,
            tc.tile_pool(name="et", bufs=et_bufs) as ep,
            tc.tile_pool(name="ps_s", bufs=2, space="PSUM") as ps_s,
            tc.tile_pool(name="ps_u", bufs=1, space="PSUM") as ps_u,
        ):
            def proj_tile():
                return ps_s.tile([128, 1024], F32, name="s", tag="s")

            # ---------- persistent tiles ----------
            w_q = [pp.tile([128, E], F32R, name=f"wq{d}", tag=f"wq{d}") for d in range(4)]
            w_k = [pp.tile([128, E], BF16, name=f"wk{d}", tag=f"wk{d}") for d in range(4)]
            w_v = [pp.tile([128, E], BF16, name=f"wv{d}", tag=f"wv{d}") for d in range(4)]
            w_oh = [pp.tile([DH, D], F32R, name=f"wo{h}", tag=f"wo{h}") for h in range(4)]
            ones64 = pp.tile([1, DH], F32R, name="ones64", tag="ones64")
            ones64f = pp.tile([1, DH], F32, name="ones64f", tag="ones64f")
            x_t = [pp.tile([128, NI], F32R, name=f"x{d}", tag=f"x{d}") for d in range(4)]
            kT = pp.tile([128, 2, NJ], BF16, name="kT", tag="kT")
            v_sb = pp.tile([128, NJC * 4 * VW], BF16, name="v_sb", tag="v_sb")
            qT = [pp.tile([128, NI], BF16, name=f"qT{e}", tag=f"qT{e}") for e in range(2)]

            # DMA order: k-proj inputs first (first block's critical path),
            # then q-proj inputs, then v weights
            for d in range(4):
                rows = slice(d * 128, (d + 1) * 128)
                nc.sync.dma_start(w_k[d][:], wkB_d[rows, :])
            cxb_pre = [sp.tile([128, 512], BF16, name=f"cxb{d}", tag=f"cxb{d}")
                       for d in range(4)]
            for d in range(4):
                nc.sync.dma_start(cxb_pre[d][:], ctxB_d[d * 128:(d + 1) * 128, 0:512])
            for d in range(4):
                rows = slice(d * 128, (d + 1) * 128)
                nc.sync.dma_start(w_q[d][:], wqT_d[rows, :].bitcast(F32R))
                nc.sync.dma_start(x_t[d][:], xT_d[rows, :].bitcast(F32R))
            for d in range(4):
                rows = slice(d * 128, (d + 1) * 128)
                nc.sync.dma_start(w_v[d][:], wvB_d[rows, :])
            # ones columns interleaved in v (softmax denominator trick)
            nc.vector.memset(v_sb[:, DH:NJC * 4 * VW:VW], 1.0)
            nc.vector.memset(ones64f[:], 1.0)
            nc.vector.tensor_copy(ones64[:], ones64f[:])

            def q_proj():
                # ---------- q projection: qT[e, i] ----------
                pq = proj_tile()
                for ec in range(2):
                    for d in range(4):
                        nc.tensor.matmul(
                            pq[:, ec * 512:(ec + 1) * 512],
                            w_q[d][:, ec * 128:(ec + 1) * 128], x_t[d][:],
                            start=(d == 0), stop=(d == 3),
                        )
                nc.vector.tensor_copy(qT[0][:], pq[:, 0:512])
                nc.scalar.copy(qT[1][:], pq[:, 512:1024])

            def do_exp(et, s_ps, J, w):
                # wave 0 goes to the Vector engine (starts earlier), wave 1
                # to Act; every act_every-th chunk Act takes both (balance).
                if dve_exp and w == 0 and (J % act_every != act_every - 1):
                    nc.vector._custom_dve(
                        exp_op, out=et.bitcast(I16), in0=s_ps,
                        s0=128.0 * LOG2E * SCALE, s1=128.0 * (127.0 - SCH_C),
                    )
                else:
                    nc.scalar.activation(et, s_ps, EXP, scale=SCALE)

            def att_scores(J):
                """scores + exp for both waves of chunk J; returns et tiles."""
                ets = []
                for w in range(2):
                    s_ps = ps_s.tile([128, 1024], F32, name="s", tag="s")
                    for p in range(2):
                        nc.tensor.matmul(
                            s_ps[:, p * 512:(p + 1) * 512],
                            kT[p * 64:(p + 1) * 64, w, J * 128:(J + 1) * 128],
                            qT[w][p * 64:(p + 1) * 64, :],
                        )
                    et = ep.tile([128, 1024], BF16, name="et", tag="et")
                    do_exp(et[:], s_ps[:], J, w)
                    ets.append(et)
                return ets

            def att_av(J, ets):
                for w in range(2):
                    for p in range(2):
                        h = 2 * w + p
                        nc.tensor.matmul(
                            U_all[0:VW, h * 512:(h + 1) * 512],
                            v_sb[:, (J * 4 + h) * VW:(J * 4 + h + 1) * VW],
                            ets[w][:, p * 512:(p + 1) * 512],
                            start=(J == 0), stop=(J == NJC - 1),
                            skip_group_check=True,
                        )

            # U[h]: [0:64, h*512:+512] = unnormalized attn out (e, i);
            # row 64 = softmax denominator
            U_all = ps_u.tile([128, 2048], F32, name="u_all", tag="u_all")

            # ---------- main loop over j-blocks ----------
            prev_av = None
            for _rep in range(reps):
              for jb in range(NBLK):
                  if jb == 1 and _rep == 0:
                      for h in range(4):
                          nc.sync.dma_start(w_oh[h][:], woT_d[h].bitcast(F32R))
                  if jb == 0 and _rep == 0:
                      cxb = cxb_pre
                  else:
                      cxb = [sp.tile([128, 512], BF16, name=f"cxb{d}", tag=f"cxb{d}")
                             for d in range(4)]
                      for d in range(4):
                          nc.sync.dma_start(
                              cxb[d][:],
                              ctxB_d[d * 128:(d + 1) * 128, jb * 512:(jb + 1) * 512],
                          )

                  def proj_piece(jc):
                      # independent PE work emitted between a chunk's exps and
                      # its A*V matmuls (fills the exp wait in the FIFO queue)
                      if jc == 0:
                          # k projection: kT[e, w, j] (both ec halves)
                          proj_piece.pk = proj_tile()
                          pk = proj_piece.pk
                          for ec in range(2):
                              for d in range(4):
                                  nc.tensor.matmul(
                                      pk[:, ec * 512:(ec + 1) * 512],
                                      w_k[d][:, ec * 128:(ec + 1) * 128], cxb[d][:],
                                      start=(d == 0), stop=(d == 3),
                                  )
                          kdst = kT[:, :, jb * 512:(jb + 1) * 512]
                          if jb % 2 == 0:
                              nc.vector.tensor_copy(
                                  kdst, pk[:, 0:1024].rearrange("p (a b) -> p a b", a=2))
                          else:
                              nc.scalar.copy(
                                  kdst, pk[:, 0:1024].rearrange("p (a b) -> p a b", a=2))
                      elif jc == 2:
                          # v projection for all 4 chunks of this block
                          proj_piece.pv = proj_tile()
                          pv = proj_piece.pv
                          for vjc in range(4):
                              for d in range(4):
                                  nc.tensor.matmul(
                                      pv[:, vjc * 256:(vjc + 1) * 256],
                                      cxb[d][:, vjc * 128:(vjc + 1) * 128], w_v[d][:],
                                      start=(d == 0), stop=(d == 3),
                                  )
                      elif jc == 3:
                          pv = proj_piece.pv
                          for half in range(2):
                              Jv = jb * 4 + half * 2
                              vdst = v_sb[:, Jv * 4 * VW:(Jv + 2) * 4 * VW]
                              vdst = vdst.rearrange(
                                  "p (c h w) -> p c h w", c=2, w=VW)[:, :, :, 0:DH]
                              vsrc = pv[:, half * 512:(half + 1) * 512].rearrange(
                                  "p (c h d) -> p c h d", c=2, h=4)
                              if half == 0:
                                  nc.vector.tensor_copy(vdst, vsrc)
                              else:
                                  nc.scalar.copy(vdst, vsrc)

                  if jb == 0:
                      proj_piece(0)
                      if _rep == 0:
                          q_proj()
                      proj_piece(1)
                      proj_piece(2)
                      proj_piece(3)
                  else:
                      # software-pipelined: A*V for chunk J-1 is emitted after
                      # chunk J's scores+exp and the projection filler, so its
                      # wait on the exp engines is fully hidden.
                      ab = jb - 1
                      for jc in range(4):
                          J = ab * 4 + jc
                          ets = att_scores(J)
                          proj_piece(jc)
                          if prev_av is not None:
                              att_av(J - 1, prev_av)
                          prev_av = ets

              # drain: attention for the final block
              for jc in range(4):
                  J = (NBLK - 1) * 4 + jc
                  ets = att_scores(J)
                  if prev_av is not None:
                      att_av(J - 1, prev_av)
                  prev_av = ets
              att_av(NJC - 1, prev_av)
              prev_av = None

              if not rep_epilogue and _rep != reps - 1:
                  continue
              # ---------- epilogue: per-head normalize + output projection ----
              # breadth-first across heads so the engine FIFOs pipeline the
              # per-head chains instead of head-of-line blocking on each other
              un = [pp.tile([DH, NI], F32R, name=f"un{h}", tag=f"un{h}") for h in range(4)]
              recs, rec_rs, rbs, rb_sbs = [], [], [], []
              r_hs = []
              for h in range(4):
                  r_h = pp.tile([1, NI], F32, name=f"r{h}", tag=f"r{h}")
                  if h % 2 == 0:
                      nc.vector.tensor_copy(r_h[:], U_all[DH:DH + 1, h * 512:(h + 1) * 512])
                  else:
                      nc.scalar.copy(r_h[:], U_all[DH:DH + 1, h * 512:(h + 1) * 512])
                  r_hs.append(r_h)
              for h in range(4):
                  rec = pp.tile([1, NI], F32, name=f"rec{h}", tag=f"rec{h}")
                  nc.vector.reciprocal_approx_fast(rec[:], r_hs[h][:])
                  recs.append(rec)
              for h in range(4):
                  rec_r = pp.tile([1, NI], F32R, name=f"recr{h}", tag=f"recr{h}")
                  if h % 2 == 0:
                      nc.scalar.copy(rec_r[:], recs[h][:])
                  else:
                      nc.vector.tensor_copy(rec_r[:], recs[h][:])
                  rec_rs.append(rec_r)
              for pair in range(2):
                  for h in (2 * pair, 2 * pair + 1):
                      rb = proj_tile()
                      # K=1 matmul broadcasts the reciprocal row to 64 partitions
                      nc.tensor.matmul(rb[0:DH, 0:512], ones64[:], rec_rs[h][:])
                      rbs.append(rb)
                  for h in (2 * pair, 2 * pair + 1):
                      rb_sb = pp.tile([DH, NI], F32, name=f"rb_sb{h}", tag=f"rb_sb{h}")
                      if h % 2 == 0:
                          nc.scalar.copy(rb_sb[:], rbs[h][0:DH, 0:512])
                      else:
                          nc.vector.tensor_copy(rb_sb[:], rbs[h][0:DH, 0:512])
                      rb_sbs.append(rb_sb)
              for h in range(4):
                  nc.vector.tensor_mul(un[h][:], U_all[0:DH, h * 512:(h + 1) * 512], rb_sbs[h][:])
              for ic in range(4):
                  po = proj_tile()
                  for h in range(4):
                      nc.tensor.matmul(
                          po[:, 0:512], un[h][:, ic * 128:(ic + 1) * 128], w_oh[h][:],
                          start=(h == 0), stop=(h == 3),
                      )
                  o_sb = pp.tile([128, D], F32, name=f"o{ic}", tag=f"o{ic}")
                  if ic % 2 == 0:
                      nc.vector.tensor_copy(o_sb[:], po[:, 0:512])
                  else:
                      nc.scalar.copy(o_sb[:], po[:, 0:512])
                  nc.sync.dma_start(out_d[ic * 128:(ic + 1) * 128, :], o_sb[:])

    nc.compile()
    return nc


def make_in_maps(inputs):
    x = np.asarray(inputs["x"], dtype=np.float32)
    context = np.asarray(inputs["context"], dtype=np.float32)
    Wq = np.asarray(inputs["Wq"], dtype=np.float32)
    Wkv = np.asarray(inputs["Wkv"], dtype=np.float32)
    Wout = np.asarray(inputs["Wout"], dtype=np.float32)
    in_maps = []
    for b in range(B):
        ctxT = np.ascontiguousarray(np.concatenate([x[b], context[b]], axis=0).T)
        ctxB = ctxT.astype(ml_dtypes.bfloat16)
        xT = np.ascontiguousarray(x[b].T)
        for g in range(2):
            sl = slice(g * E, (g + 1) * E)
            # woT[h] = Wout[:, g*256 + h*64 : +64].T  -> [64, 512]
            woT = np.ascontiguousarray(Wout[:, sl].T.reshape(4, DH, D))
            in_maps.append({
                "xT": xT,
                "ctxB": ctxB,
                "wqT": np.ascontiguousarray(Wq[sl, :].T),
                "wkB": np.ascontiguousarray(Wkv[sl, :].T).astype(ml_dtypes.bfloat16),
                "wvB": np.ascontiguousarray(
                    Wkv[D + g * E:D + (g + 1) * E, :].T).astype(ml_dtypes.bfloat16),
                "woT": woT,
            })

    return in_maps


def kernel(**inputs):
    if "nc" not in _CACHE:
        _CACHE["nc"] = _build_nc()
    nc = _CACHE["nc"]
    in_maps = make_in_maps(inputs)
    res = bass_utils.run_bass_kernel_spmd(nc, in_maps, core_ids=list(range(8)))
    outs = [r["out"] for r in res.results]
    final = np.empty((B, NI, D), np.float32)
    for b in range(B):
        final[b] = outs[2 * b] + outs[2 * b + 1]
    return final
